# revision 65
# baseline (speedup 1.0000x reference)
"""MoE update-MLP Trainium2 kernel (8-core SPMD, sparse top-2 expert compute).

Problem: x (4,192,128,128); a per-pixel router picks top-2 of 8 experts; each
expert is a 3-layer 1x1-conv MLP (192->384 gelu ->384 gelu ->192); output is
the gate-weighted sum over experts.

Strategy: the router is a tiny K=8 linear layer (0.005% of the FLOPs) --
computed on the host, which packs only the top-2 (pixel, expert) assignments
into per-core, per-expert contiguous segments (capacity = largest per-core
chunk, rounded up to even). Each of the 8 cores runs a pure dense GEMM stack
over its ~16.4k assigned pixel-slots (vs 65.5k expert-pixel pairs dense).
The host then applies gates and scatter-adds each pixel's two expert outputs
(plus the gated b3 term) into the full output.

Everything streams in bf16 (x, W1-3, h1, h2, staged outputs; fp32 PSUM
accumulation) -- measured rel err 4.2e-3 vs the 2e-2 budget. Per 512-wide
pixel tile the schedule is 21 uniform full-array 128x128 matmuls (L1: 2
K-chunks x 3 M-chunks with the K=192 contraction zero-padded to 256; L2:
3x3; L3: 3 K-chunks x 2 M-chunks with OUT_C zero-padded 192->256).
Partial-array (tile_position row/col-tiled) variants were measured SLOWER
overall: a tiled LDWEIGHTS cannot preload into the PE background weight
buffer, so every tiled<->full boundary costs ~95-230ns, cancelling the
saved slots; uniform full-array matmuls keep the LDWEIGHTS pipeline
perfect at ~216ns/MM (N=512 streaming rate).

Measured fixed overheads handled explicitly:
- ~6.6us framework preamble; a ~4us chain of dummy matmuls on a memset
  SBUF tile right after it keeps the PE busy through the HAM activity
  window while the first DMAs land, so real matmuls start at the warm
  2.4 GHz clock (cold clock is 1.2 GHz for the first ~3.4us of activity,
  and the first data cannot arrive before ~10us).
- A dummy fp32-input gelu activation pulls the ~2x1.3us ACT_TABLE_LOAD
  off the critical path. Its table transfers ride the scalar HWDGE ring,
  so no startup-critical DMA may use nc.scalar.dma_start.
- Startup DMAs: x tiles + biases on the sync ring, expert-0 weights in
  pieces (first-needed blocks first) + later experts (one DMA each, a
  full segment ahead) on the gpsimd ring.
- Outputs stage into per-flush-chunk SBUF tiles (bufs=3) so tensor_copies
  never wait on an in-flight flush DMA reading the same tile; chunks are
  half-segments, the last expert flushes per tile and orders its narrow
  tile last, and the final tile's two copies/flushes split across the
  scalar+vector engines and scalar+sync rings to shorten the
  copy->flush->drain serial tail.

Software pipeline per tile i: [L2(i) -> gelu] [L1(i+1) -> gelu] [L3(i) ->
copy], with x loads 2 tiles ahead, so ACT latency hides under PE work.
PSUM: 3 (L1) + 3 (L2) + 2 (L3) banks = 8.
"""

import ml_dtypes
import numpy as np

import concourse.bacc as bacc
import concourse.mybir as mybir
import concourse.tile as tile
from concourse.bass_utils import run_bass_kernel_spmd

F32 = mybir.dt.float32
F32R = mybir.dt.float32r
BF16 = mybir.dt.bfloat16
AF = mybir.ActivationFunctionType

N_CORES = 8
B, IN_C, H, W = 4, 192, 128, 128
R_C, E, HID, OUT_C = 8, 8, 384, 192
NPIX = B * H * W
TILE = 512
WARMUP_MMS = 9

_nc_cache: dict = {}


def _tile_seq(caps):
    """[(expert, col_start, width)] covering each expert's capacity segment.

    Tiles are 512 wide except for an odd tail kept >=256 (fp32r matmuls
    below free-dim 256 run at 1/4 rate); a tail under 256 borrows from the
    last full tile. Narrow tiles lead each segment so the very first tile
    of the program needs the least DMA before compute can start -- except
    the last expert, whose narrow tile goes last so the final output flush
    (which the end-of-program drain waits on) is as small as possible.
    """
    seq, off = [], 0
    for e, cap in enumerate(caps):
        k, t = divmod(cap, TILE)
        if t == 0:
            widths = [TILE] * k
        elif t >= 256 or k == 0:
            widths = [max(t, 256)] + [TILE] * k
        else:
            a = (((TILE + t) // 2) + 1) & ~1
            widths = [a, TILE + t - a] + [TILE] * (k - 1)
        if e == E - 1:
            widths = widths[::-1]
        o = 0
        for w in widths:
            seq.append((e, off + o, w))
            o += w
        off += cap
    return seq


def _build(caps, zero_bias: bool = False, compile: bool = True):
    nslot = sum(caps)
    nc = bacc.Bacc("TRN2", target_bir_lowering=False, debug=False)

    # x and W1 stream in bf16 (the verifier requires both matmul operands
    # to match when either is f32/f32r): halves the largest DMA stream at
    # the same PE rate, with fp32 PSUM accumulation. Only layer 1's inputs
    # are rounded (~0.4%), well inside the 2e-2 budget. Slab 1 of xp/w1t
    # holds channels 128:192 in partitions 0:64 AND replicated in 64:128,
    # so the K=64 remainder matmuls can be row-tiled to either half.
    # w1 | w2 | w3 concatenated, one row per expert; w3's 192 output
    # channels are padded per k-chunk to 2x128 (zero cols) so every matmul
    # is a full-array 128x128 (partial-array LDWEIGHTS cannot preload into
    # the background weight buffer and costs ~95ns per group boundary)
    WCOLS = 2 * HID + 3 * HID + 3 * 256
    W2OFF = 2 * HID
    W3OFF = 2 * HID + 3 * HID
    xp_in = nc.declare_dram_parameter("xp", [128, 2, nslot], BF16, isOutput=False)
    wall_in = nc.declare_dram_parameter("wall", [E, 128, WCOLS], BF16, isOutput=False)
    b12_in = nc.declare_dram_parameter("b12t", [128, 2, E * 3], F32, isOutput=False)
    # output staged and stored as bf16: halves the out DMA stream and the
    # final flush the end-of-program drain waits on; the host upcasts.
    # yp[:, 0] = out channels 0:128; yp[:, 1] rows 0:64 / 64:128 hold the
    # two col-tiled partial sums of channels 128:192 (host adds them).
    yp_out = nc.declare_dram_parameter("yp", [128, 2, nslot], BF16, isOutput=True)

    seq = _tile_seq(caps)
    nt = len(seq)

    with tile.TileContext(nc) as tc:
        with (
            tc.tile_pool(name="wpool", bufs=1) as wpool,
            tc.tile_pool(name="xpool", bufs=4) as xpool,
            tc.tile_pool(name="hpool", bufs=6) as hpool,
            tc.tile_pool(name="psp", bufs=3, space="PSUM") as psp,
        ):
            opool = hpool
            ps1p = ps2p = ps3p = psp
            b12_sb = wpool.tile([128, 2, E * 3], F32)
            b1_sb = b12_sb[:, 0]
            b2_sb = b12_sb[:, 1]
            w_all = wpool.tile([128, E, WCOLS], BF16)

            def w1_blk(e, k, m):  # [128, 128] k-chunk x m-chunk of W1^T
                return w_all[:, e, k * HID + 128 * m : k * HID + 128 * (m + 1)]

            def w2_blk(e, k, m):
                return w_all[
                    :, e, W2OFF + k * HID + 128 * m : W2OFF + k * HID + 128 * (m + 1)
                ]

            def w3_blk(e, k, m):  # m=0: out 0:128; m=1: out 128:192 + 64 zero
                return w_all[
                    :, e, W3OFF + k * 256 + 128 * m : W3OFF + k * 256 + 128 * (m + 1)
                ]

            # PE warmup: a chain of matmuls on a zeroed SBUF tile with no
            # DMA dependency keeps the PE busy right after the framework
            # preamble, so the HAM clock gate is at 8/8 by the time the
            # first data arrives. A dummy activation pulls the ~1.3us gelu
            # ACT_TABLE_LOAD off the critical path too.
            dummy = wpool.tile([128, 128 + TILE], BF16)
            dummy_f = wpool.tile([128, 8], F32)
            dscr = wpool.tile([128, 8], BF16)
            nc.vector.memset(dummy[:], 0.0)
            nc.vector.memset(dummy_f[:], 0.0)

            def load_x(i, split=False):
                _, s, wd = seq[i]
                xs = xpool.tile([128, 2, TILE], BF16, tag="xs", name=f"xs_{i}")
                if split:
                    # chunk 0 lands first so the first three matmuls (which
                    # only need channels 0:128) start as early as possible.
                    # All on the sync queue: the scalar queue is polluted by
                    # the ~1.3us gelu ACT_TABLE_LOAD transfers early on.
                    nc.sync.dma_start(xs[:, 0, :wd], xp_in[:, 0, s : s + wd])
                    nc.sync.dma_start(xs[:, 1, :wd], xp_in[:, 1, s : s + wd])
                else:
                    nc.sync.dma_start(xs[:, :, :wd], xp_in[:, :, s : s + wd])
                return xs

            def l1(i, xs):
                e, _, wd = seq[i]
                h1 = []
                pss = [
                    ps1p.tile([128, TILE], F32, tag="ps1", name=f"ps1_{i}_{m}")
                    for m in range(3)
                ]
                for m in range(3):
                    nc.tensor.matmul(
                        pss[m][:, :wd],
                        w1_blk(e, 0, m),
                        xs[:, 0, :wd],
                        start=True,
                        stop=False,
                    )
                # K remainder: channels 128:192 + 64 zero weight rows, as
                # full-array matmuls (uniform full-array MMs keep the
                # LDWEIGHTS pipeline perfect)
                for m in range(3):
                    nc.tensor.matmul(
                        pss[m][:, :wd],
                        w1_blk(e, 1, m),
                        xs[:, 1, :wd],
                        start=False,
                        stop=True,
                    )
                for m in range(3):
                    hm = hpool.tile([128, TILE], BF16, tag="h1", name=f"h1_{i}_{m}")
                    nc.scalar.activation(
                        hm[:, :wd],
                        pss[m][:, :wd],
                        AF.Gelu,
                        bias=b1_sb[:, 3 * e + m : 3 * e + m + 1],
                    )
                    h1.append(hm)
                return h1

            def l2(i, h1):
                e, _, wd = seq[i]
                # ps2 split 2+1 banks: with zero biases the m0+m1 gelu runs
                # as ONE ACTIVATE over both banks, cutting the scalar
                # engine's per-ACT 352-cycle fixed overhead (scalar is
                # otherwise co-critical with the PE on narrow tiles). h1
                # stays per-m: its ACTs queue behind h2's on the scalar
                # FIFO, and merging them too delays h1[0] past its
                # consumer (measured).
                pa2 = ps2p.tile(
                    [128, 2, TILE], F32, tag="ps2a", bufs=1, name=f"ps2a_{i}"
                )
                pb1 = ps2p.tile(
                    [128, TILE], F32, tag="ps2b", bufs=1, name=f"ps2b_{i}"
                )
                pss = [pa2[:, 0], pa2[:, 1], pb1]
                for k in range(3):
                    for m in range(3):
                        nc.tensor.matmul(
                            pss[m][:, :wd],
                            w2_blk(e, k, m),
                            h1[k][:, :wd],
                            start=(k == 0),
                            stop=(k == 2),
                        )
                h2 = hpool.tile([128, 3, TILE], BF16, tag="h2", name=f"h2_{i}")
                if zero_bias:
                    nc.scalar.activation(h2[:, 0:2, :wd], pa2[:, :, :wd], AF.Gelu)
                    nc.scalar.activation(h2[:, 2, :wd], pb1[:, :wd], AF.Gelu)
                else:
                    for m in range(3):
                        nc.scalar.activation(
                            h2[:, m, :wd],
                            pss[m][:, :wd],
                            AF.Gelu,
                            bias=b2_sb[:, 3 * e + m : 3 * e + m + 1],
                        )
                return h2

            # flush chunks: tiles are staged into per-chunk SBUF tiles and
            # flushed when the chunk completes, so later tensor_copies never
            # wait on an in-flight flush DMA reading the same tile. Chunks
            # split each expert's segment at the half-capacity tile; the
            # last expert flushes per tile so the end-of-program drain only
            # waits on one small transfer.
            chunk_of, chunk_start, chunk_end = {}, {}, {}
            for e0 in range(E):
                tiles_e = [j for j in range(nt) if seq[j][0] == e0]
                groups = []
                if e0 == E - 1:
                    groups = [[j] for j in tiles_e]
                else:
                    cur = []
                    for j in tiles_e:
                        cur.append(j)
                        _, s0, wd0 = seq[j]
                        off0 = seq[tiles_e[0]][1]
                        if s0 + wd0 - off0 >= caps[e0] // 2 and len(groups) == 0:
                            groups.append(cur)
                            cur = []
                    if cur:
                        groups.append(cur)
                for g in groups:
                    st = seq[g[0]][1]
                    en = seq[g[-1]][1] + seq[g[-1]][2]
                    for j in g:
                        chunk_of[j] = g[0]
                        chunk_start[j] = st
                        chunk_end[j] = en
            chunk_max = max(
                chunk_end[j] - chunk_start[j] for j in range(nt)
            )
            oseg = {"o": None}

            def l3(i, h2):
                e, s, wd = seq[i]
                pa = ps3p.tile([128, TILE], F32, tag="oa", bufs=1, name=f"oa_{i}")
                pb = ps3p.tile([128, TILE], F32, tag="ob", bufs=1, name=f"ob_{i}")
                # interleave the two PSUM banks so no matmul accumulates
                # into the bank written by the immediately preceding one;
                # pb's weight block is zero-padded 192->256 so it is a
                # full-array matmul too (rows 64:128 of pb are zeros)
                for k in range(3):
                    nc.tensor.matmul(
                        pa[:, :wd],
                        w3_blk(e, k, 0),
                        h2[:, k, :wd],
                        start=(k == 0),
                        stop=(k == 2),
                    )
                    nc.tensor.matmul(
                        pb[:, :wd],
                        w3_blk(e, k, 1),
                        h2[:, k, :wd],
                        start=(k == 0),
                        stop=(k == 2),
                    )
                if chunk_of[i] == i:
                    oseg["o"] = opool.tile(
                        [128, 2, chunk_max], BF16, tag="os", bufs=3, name=f"os_{i}"
                    )
                os = oseg["o"]
                o = s - chunk_start[i]
                lastp = i + 1 == nt
                if lastp:
                    # last tile of the program: pa copies on the (now idle)
                    # scalar engine while the vector engine copies pb, and
                    # the two flush halves ride different queues, to
                    # shorten the copy->flush->drain serial tail
                    nc.scalar.copy(os[:, 0, o : o + wd], pa[:, :wd])
                    nc.vector.tensor_copy(os[0:64, 1, o : o + wd], pb[0:64, :wd])
                else:
                    nc.vector.tensor_copy(os[:, 0, o : o + wd], pa[:, :wd])
                    nc.vector.tensor_copy(os[0:64, 1, o : o + wd], pb[0:64, :wd])
                if i + 1 == nt or chunk_of[i + 1] != chunk_of[i]:
                    lo, hi = chunk_start[i], chunk_end[i]
                    # out flushes ride the lightly-loaded sync queue so the
                    # gpsimd drain chain only carries the early weight DMAs
                    if lastp:
                        nc.sync.dma_start(
                            yp_out[:, 0, lo:hi], os[:, 0, : hi - lo]
                        )
                        nc.scalar.dma_start(
                            yp_out[:, 1, lo:hi], os[:, 1, : hi - lo]
                        )
                    else:
                        nc.sync.dma_start(
                            yp_out[:, :, lo:hi], os[:, :, : hi - lo]
                        )

            def load_w(e):
                nc.gpsimd.dma_start(w_all[:, e], wall_in[e])

            # Startup DMA routing, ordered for the first tile's critical
            # path: expert 0's w1 pieces ride the sync queue right behind
            # the (small) first x piece; the larger w2/w3 pieces go on
            # gpsimd in parallel; the rest of the first x tile rides the
            # scalar queue. Biases are needed only by the first ACTIVATE.
            # The sync queue is the x-tile lifeline: the early x tiles are
            # consumed just-in-time, so nothing else may ride ahead of
            # them (measured: +96KB inserted before xs1 costs ~9us of
            # head stalls). Expert-0 weights go on gpsimd in 4 pieces --
            # more pieces lose more to the ~0.65us SWDGE per-DMA issue
            # overhead at the cold start.
            xs_cur = load_x(0, split=True)
            nc.sync.dma_start(b12_sb[:], b12_in[:])
            xs_next = load_x(1) if nt > 1 else None
            for a, b in [(0, HID), (HID, W2OFF), (W2OFF, W3OFF), (W3OFF, WCOLS)]:
                nc.gpsimd.dma_start(w_all[:, 0, a:b], wall_in[0, :, a:b])
            if E > 1:
                load_w(1)
            # warmup chain (no data deps beyond the memset above); the
            # dummy activation forces the gelu table load early
            wps = ps3p.tile([128, TILE], F32, tag="oa", bufs=1, name="warm_ps")
            nc.scalar.activation(dscr[:], dummy_f[:], AF.Gelu)
            for _ in range(WARMUP_MMS):
                nc.tensor.matmul(
                    wps[:], dummy[:, 0:128], dummy[:, 128:], start=True, stop=True
                )
            h1_cur = l1(0, xs_cur)
            for i in range(nt):
                if i and seq[i][0] != seq[i - 1][0]:
                    nxt = seq[i][0] + 1
                    if nxt < E:
                        load_w(nxt)
                h2 = l2(i, h1_cur)
                if i + 1 < nt:
                    h1_cur = l1(i + 1, xs_next)
                    xs_next = load_x(i + 2) if i + 2 < nt else None
                l3(i, h2)

    if compile:
        nc.compile()
    return nc


def _get_nc(caps, zero_bias):
    key = (tuple(caps), zero_bias)
    if key not in _nc_cache:
        _nc_cache[key] = _build(tuple(caps), zero_bias)
    return _nc_cache[key]


def _route(router_input, router_W, router_b):
    """Replicate reference _gates selection: top-2 by value, 2-way softmax."""
    r = (
        np.asarray(router_input, np.float32)
        .transpose(1, 0, 2, 3)
        .reshape(R_C, NPIX)
    )
    lt = (np.asarray(router_W, np.float32) @ r).T + np.asarray(
        router_b, np.float32
    )[None, :]
    ar = np.arange(NPIX)
    i1 = np.argmax(lt, axis=1)
    l1v = lt[ar, i1]
    ltm = lt.copy()
    ltm[ar, i1] = -np.inf
    i2 = np.argmax(ltm, axis=1)
    l2v = lt[ar, i2]
    e2 = np.exp(l2v - l1v)
    g1 = (1.0 / (1.0 + e2)).astype(np.float32)
    g2 = (e2 / (1.0 + e2)).astype(np.float32)
    return i1, i2, g1, g2


def _plan(i1, i2):
    """Pack (pixel, expert) assignments into per-core per-expert segments.

    Returns caps (per-expert capacity), sl_pix
    [N_CORES, nslot] gather map (pixel index per slot, 0 for padding), and
    M [NPIX, E] with the global flat slot id (core*nslot + slot) of each
    real assignment.
    """
    pe_list, sizes_list = [], []
    caps = []
    for e in range(E):
        pe = np.flatnonzero((i1 == e) | (i2 == e))
        n = len(pe)
        base, r = divmod(n, N_CORES)
        sizes = [base + 1] * r + [base] * (N_CORES - r)
        # max chunk size rounded up to even (fp32r matmul free-dim
        # restriction); floor 256 keeps every tile >=256 wide
        caps.append(max(256, (max(sizes) + 1) & ~1))
        pe_list.append(pe)
        sizes_list.append(sizes)
    nslot = sum(caps)
    offs = np.concatenate([[0], np.cumsum(caps)])[:E]
    sl_pix = np.zeros((N_CORES, nslot), np.int64)
    M = np.zeros((NPIX, E), np.int64)
    for e in range(E):
        pe, sizes = pe_list[e], sizes_list[e]
        start = 0
        for c in range(N_CORES):
            chunk = pe[start : start + sizes[c]]
            start += sizes[c]
            sl_pix[c, offs[e] : offs[e] + len(chunk)] = chunk
            M[chunk, e] = c * nslot + offs[e] + np.arange(len(chunk))
    return caps, sl_pix, M


def kernel(x, router_input, router_W, router_b, W1, b1, W2, b2, W3, b3, **run_kwargs):
    f = np.float32
    i1, i2, g1, g2 = _route(router_input, router_W, router_b)
    caps, sl_pix, M = _plan(i1, i2)
    zero_bias = bool(
        np.all(np.asarray(b1) == 0.0) and np.all(np.asarray(b2) == 0.0)
    )
    nc = _get_nc(caps, zero_bias)

    x_flat = np.asarray(x, f).transpose(1, 0, 2, 3).reshape(IN_C, NPIX)
    w1T = np.transpose(np.asarray(W1, f), (0, 2, 1))  # [E, IN_C, HID]
    w1t = np.zeros((E, 128, 2, HID), f)
    w1t[:, :, 0, :] = w1T[:, 0:128, :]
    w1t[:, 0:64, 1, :] = w1T[:, 128:IN_C, :]
    w1t = w1t.astype(ml_dtypes.bfloat16)
    w2t = np.transpose(np.asarray(W2, f), (0, 2, 1))
    w2t = np.ascontiguousarray(
        w2t.reshape(E, 3, 128, HID).transpose(0, 2, 1, 3)
    ).astype(ml_dtypes.bfloat16)
    w3t = np.transpose(np.asarray(W3, f), (0, 2, 1))
    w3t = np.ascontiguousarray(
        w3t.reshape(E, 3, 128, OUT_C).transpose(0, 2, 1, 3)
    )
    w3p = np.zeros((E, 128, 3, 256), np.float32)
    w3p[:, :, :, 0:OUT_C] = w3t
    w3p = w3p.astype(ml_dtypes.bfloat16)
    b1t = np.asarray(b1, f).reshape(E, 3, 128).transpose(2, 0, 1).reshape(128, E * 3)
    b2t = np.asarray(b2, f).reshape(E, 3, 128).transpose(2, 0, 1).reshape(128, E * 3)
    b12t = np.ascontiguousarray(np.stack([b1t, b2t], axis=1))
    wall = np.ascontiguousarray(
        np.concatenate(
            [
                w1t.reshape(E, 128, 2 * HID),
                w2t.reshape(E, 128, 3 * HID),
                w3p.reshape(E, 128, 3 * 256),
            ],
            axis=2,
        )
    )

    nslot = sum(caps)
    in_maps = []
    for c in range(N_CORES):
        xg = x_flat[:, sl_pix[c]]
        xp = np.zeros((128, 2, nslot), ml_dtypes.bfloat16)
        xp[:, 0, :] = xg[0:128]
        xp[0:64, 1, :] = xg[128:IN_C]
        xp[64:128, 1, :] = xg[128:IN_C]
        in_maps.append(
            {
                "xp": xp,
                "wall": wall,
                "b12t": b12t,
            }
        )

    res = run_bass_kernel_spmd(nc, in_maps, list(range(N_CORES)), **run_kwargs)

    # yp[:, 0] = channels 0:128; yp[0:64, 1] = channels 128:192
    yp_all = np.concatenate(
        [res.results[c]["yp"] for c in range(N_CORES)], axis=2
    ).astype(f)
    yp192 = np.concatenate([yp_all[:, 0, :], yp_all[0:64, 1, :]], axis=0)
    ar = np.arange(NPIX)
    j1 = M[ar, i1]
    j2 = M[ar, i2]
    b3f = np.asarray(b3, f)
    out_flat = (
        yp192[:, j1] * g1[None, :]
        + yp192[:, j2] * g2[None, :]
        + b3f[i1].T * g1[None, :]
        + b3f[i2].T * g2[None, :]
    )
    full = np.ascontiguousarray(
        out_flat.reshape(OUT_C, B, H, W).transpose(1, 0, 2, 3).astype(f)
    )
    if run_kwargs:
        kernel.last_results = res
    return full


# revision 66
# speedup vs baseline: 1.0164x; 1.0164x over previous
"""MoE update-MLP Trainium2 kernel (8-core SPMD, sparse top-2 expert compute).

Problem: x (4,192,128,128); a per-pixel router picks top-2 of 8 experts; each
expert is a 3-layer 1x1-conv MLP (192->384 gelu ->384 gelu ->192); output is
the gate-weighted sum over experts.

Strategy: the router is a tiny K=8 linear layer (0.005% of the FLOPs) --
computed on the host, which packs only the top-2 (pixel, expert) assignments
into per-core, per-expert contiguous segments (capacity = largest per-core
chunk, rounded up to even). Each of the 8 cores runs a pure dense GEMM stack
over its ~16.4k assigned pixel-slots (vs 65.5k expert-pixel pairs dense).
The host then applies gates and scatter-adds each pixel's two expert outputs
(plus the gated b3 term) into the full output.

Everything streams in bf16 (x, W1-3, h1, h2, staged outputs; fp32 PSUM
accumulation) -- measured rel err 4.2e-3 vs the 2e-2 budget. Per 512-wide
pixel tile the schedule is 21 uniform full-array 128x128 matmuls (L1: 2
K-chunks x 3 M-chunks with the K=192 contraction zero-padded to 256; L2:
3x3; L3: 3 K-chunks x 2 M-chunks with OUT_C zero-padded 192->256).
Partial-array (tile_position row/col-tiled) variants were measured SLOWER
overall: a tiled LDWEIGHTS cannot preload into the PE background weight
buffer, so every tiled<->full boundary costs ~95-230ns, cancelling the
saved slots; uniform full-array matmuls keep the LDWEIGHTS pipeline
perfect at ~216ns/MM (N=512 streaming rate).

Measured fixed overheads handled explicitly:
- ~6.6us framework preamble; a ~4us chain of dummy matmuls on a memset
  SBUF tile right after it keeps the PE busy through the HAM activity
  window while the first DMAs land, so real matmuls start at the warm
  2.4 GHz clock (cold clock is 1.2 GHz for the first ~3.4us of activity,
  and the first data cannot arrive before ~10us).
- A dummy fp32-input gelu activation pulls the ~2x1.3us ACT_TABLE_LOAD
  off the critical path. Its table transfers ride the scalar HWDGE ring,
  so no startup-critical DMA may use nc.scalar.dma_start.
- Startup DMAs: x tiles + biases on the sync ring, expert-0 weights in
  pieces (first-needed blocks first) + later experts (one DMA each, a
  full segment ahead) on the gpsimd ring.
- Outputs stage into per-flush-chunk SBUF tiles (bufs=3) so tensor_copies
  never wait on an in-flight flush DMA reading the same tile; chunks are
  half-segments, the last expert flushes per tile and orders its narrow
  tile last, and the final tile's two copies/flushes split across the
  scalar+vector engines and scalar+sync rings to shorten the
  copy->flush->drain serial tail.

Software pipeline per tile i: [L2(i) -> gelu] [L1(i+1) -> gelu] [L3(i) ->
copy], with x loads 2 tiles ahead, so ACT latency hides under PE work.
PSUM: 3 (L1) + 3 (L2) + 2 (L3) banks = 8.
"""

import ml_dtypes
import numpy as np

import concourse.bacc as bacc
import concourse.mybir as mybir
import concourse.tile as tile
from concourse.bass_utils import run_bass_kernel_spmd

F32 = mybir.dt.float32
F32R = mybir.dt.float32r
BF16 = mybir.dt.bfloat16
AF = mybir.ActivationFunctionType

N_CORES = 8
B, IN_C, H, W = 4, 192, 128, 128
R_C, E, HID, OUT_C = 8, 8, 384, 192
NPIX = B * H * W
TILE = 512
WARMUP_MMS = 9

_nc_cache: dict = {}


def _tile_seq(caps):
    """[(expert, col_start, width)] covering each expert's capacity segment.

    Tiles are 512 wide except for an odd tail kept >=256 (fp32r matmuls
    below free-dim 256 run at 1/4 rate); a tail under 256 borrows from the
    last full tile. Narrow tiles lead each segment so the very first tile
    of the program needs the least DMA before compute can start -- except
    the last expert, whose narrow tile goes last so the final output flush
    (which the end-of-program drain waits on) is as small as possible.
    """
    seq, off = [], 0
    for e, cap in enumerate(caps):
        k, t = divmod(cap, TILE)
        if t == 0:
            widths = [TILE] * k
        elif t >= 256 or k == 0:
            widths = [max(t, 256)] + [TILE] * k
        else:
            a = (((TILE + t) // 2) + 1) & ~1
            widths = [a, TILE + t - a] + [TILE] * (k - 1)
        if e == E - 1:
            widths = widths[::-1]
        o = 0
        for w in widths:
            seq.append((e, off + o, w))
            o += w
        off += cap
    return seq


def _build(caps, compile: bool = True):
    nslot = sum(caps)
    nc = bacc.Bacc("TRN2", target_bir_lowering=False, debug=False)

    # x and W1 stream in bf16 (the verifier requires both matmul operands
    # to match when either is f32/f32r): halves the largest DMA stream at
    # the same PE rate, with fp32 PSUM accumulation. Only layer 1's inputs
    # are rounded (~0.4%), well inside the 2e-2 budget. Slab 1 of xp/w1t
    # holds channels 128:192 in partitions 0:64 AND replicated in 64:128,
    # so the K=64 remainder matmuls can be row-tiled to either half.
    # w1 | w2 | w3 concatenated, one row per expert; w3's 192 output
    # channels are padded per k-chunk to 2x128 (zero cols) so every matmul
    # is a full-array 128x128 (partial-array LDWEIGHTS cannot preload into
    # the background weight buffer and costs ~95ns per group boundary)
    WCOLS = 2 * HID + 3 * HID + 3 * 256
    W2OFF = 2 * HID
    W3OFF = 2 * HID + 3 * HID
    xp_in = nc.declare_dram_parameter("xp", [128, 2, nslot], BF16, isOutput=False)
    wall_in = nc.declare_dram_parameter("wall", [E, 128, WCOLS], BF16, isOutput=False)
    b12_in = nc.declare_dram_parameter("b12t", [128, 2, E * 3], F32, isOutput=False)
    # output staged and stored as bf16: halves the out DMA stream and the
    # final flush the end-of-program drain waits on; the host upcasts.
    # yp[:, 0] = out channels 0:128; yp[:, 1] rows 0:64 / 64:128 hold the
    # two col-tiled partial sums of channels 128:192 (host adds them).
    yp_out = nc.declare_dram_parameter("yp", [128, 2, nslot], BF16, isOutput=True)

    seq = _tile_seq(caps)
    nt = len(seq)

    with tile.TileContext(nc) as tc:
        with (
            tc.tile_pool(name="wpool", bufs=1) as wpool,
            tc.tile_pool(name="xpool", bufs=4) as xpool,
            tc.tile_pool(name="hpool", bufs=6) as hpool,
            tc.tile_pool(name="psp", bufs=3, space="PSUM") as psp,
        ):
            opool = hpool
            ps1p = ps2p = ps3p = psp
            b12_sb = wpool.tile([128, 2, E * 3], F32)
            b1_sb = b12_sb[:, 0]
            b2_sb = b12_sb[:, 1]
            w_all = wpool.tile([128, E, WCOLS], BF16)

            def w1_blk(e, k, m):  # [128, 128] k-chunk x m-chunk of W1^T
                return w_all[:, e, k * HID + 128 * m : k * HID + 128 * (m + 1)]

            def w2_blk(e, k, m):
                return w_all[
                    :, e, W2OFF + k * HID + 128 * m : W2OFF + k * HID + 128 * (m + 1)
                ]

            def w3_blk(e, k, m):  # m=0: out 0:128; m=1: out 128:192 + 64 zero
                return w_all[
                    :, e, W3OFF + k * 256 + 128 * m : W3OFF + k * 256 + 128 * (m + 1)
                ]

            # PE warmup: a chain of matmuls on a zeroed SBUF tile with no
            # DMA dependency keeps the PE busy right after the framework
            # preamble, so the HAM clock gate is at 8/8 by the time the
            # first data arrives. A dummy activation pulls the ~1.3us gelu
            # ACT_TABLE_LOAD off the critical path too.
            dummy = wpool.tile([128, 128 + TILE], BF16)
            dummy_f = wpool.tile([128, 8], F32)
            dscr = wpool.tile([128, 8], BF16)
            nc.vector.memset(dummy[:], 0.0)
            nc.vector.memset(dummy_f[:], 0.0)

            def load_x(i, split=False):
                _, s, wd = seq[i]
                xs = xpool.tile([128, 2, TILE], BF16, tag="xs", name=f"xs_{i}")
                if split:
                    # chunk 0 lands first so the first three matmuls (which
                    # only need channels 0:128) start as early as possible.
                    # All on the sync queue: the scalar queue is polluted by
                    # the ~1.3us gelu ACT_TABLE_LOAD transfers early on.
                    nc.sync.dma_start(xs[:, 0, :wd], xp_in[:, 0, s : s + wd])
                    nc.sync.dma_start(xs[:, 1, :wd], xp_in[:, 1, s : s + wd])
                else:
                    nc.sync.dma_start(xs[:, :, :wd], xp_in[:, :, s : s + wd])
                return xs

            def l1(i, xs):
                e, _, wd = seq[i]
                h1 = []
                pss = [
                    ps1p.tile([128, TILE], F32, tag="ps1", name=f"ps1_{i}_{m}")
                    for m in range(3)
                ]
                for m in range(3):
                    nc.tensor.matmul(
                        pss[m][:, :wd],
                        w1_blk(e, 0, m),
                        xs[:, 0, :wd],
                        start=True,
                        stop=False,
                    )
                # K remainder: channels 128:192 + 64 zero weight rows, as
                # full-array matmuls (uniform full-array MMs keep the
                # LDWEIGHTS pipeline perfect)
                for m in range(3):
                    nc.tensor.matmul(
                        pss[m][:, :wd],
                        w1_blk(e, 1, m),
                        xs[:, 1, :wd],
                        start=False,
                        stop=True,
                    )
                for m in range(3):
                    hm = hpool.tile([128, TILE], BF16, tag="h1", name=f"h1_{i}_{m}")
                    nc.scalar.activation(
                        hm[:, :wd],
                        pss[m][:, :wd],
                        AF.Gelu,
                        bias=b1_sb[:, 3 * e + m : 3 * e + m + 1],
                    )
                    h1.append(hm)
                return h1

            def l2(i, h1):
                e, _, wd = seq[i]
                pss = [
                    ps2p.tile([128, TILE], F32, tag="ps2", name=f"ps2_{i}_{m}")
                    for m in range(3)
                ]
                for k in range(3):
                    for m in range(3):
                        nc.tensor.matmul(
                            pss[m][:, :wd],
                            w2_blk(e, k, m),
                            h1[k][:, :wd],
                            start=(k == 0),
                            stop=(k == 2),
                        )
                h2 = []
                for m in range(3):
                    hm = hpool.tile([128, TILE], BF16, tag="h2", name=f"h2_{i}_{m}")
                    nc.scalar.activation(
                        hm[:, :wd],
                        pss[m][:, :wd],
                        AF.Gelu,
                        bias=b2_sb[:, 3 * e + m : 3 * e + m + 1],
                    )
                    h2.append(hm)
                return h2

            # flush chunks: tiles are staged into per-chunk SBUF tiles and
            # flushed when the chunk completes, so later tensor_copies never
            # wait on an in-flight flush DMA reading the same tile. Chunks
            # split each expert's segment at the half-capacity tile; the
            # last expert flushes per tile so the end-of-program drain only
            # waits on one small transfer.
            chunk_of, chunk_start, chunk_end = {}, {}, {}
            for e0 in range(E):
                tiles_e = [j for j in range(nt) if seq[j][0] == e0]
                groups = []
                if e0 == E - 1:
                    groups = [[j] for j in tiles_e]
                else:
                    cur = []
                    for j in tiles_e:
                        cur.append(j)
                        _, s0, wd0 = seq[j]
                        off0 = seq[tiles_e[0]][1]
                        if s0 + wd0 - off0 >= caps[e0] // 2 and len(groups) == 0:
                            groups.append(cur)
                            cur = []
                    if cur:
                        groups.append(cur)
                for g in groups:
                    st = seq[g[0]][1]
                    en = seq[g[-1]][1] + seq[g[-1]][2]
                    for j in g:
                        chunk_of[j] = g[0]
                        chunk_start[j] = st
                        chunk_end[j] = en
            chunk_max = max(
                chunk_end[j] - chunk_start[j] for j in range(nt)
            )
            oseg = {"o": None}

            def l3(i, h2):
                e, s, wd = seq[i]
                pa = ps3p.tile([128, TILE], F32, tag="oa", bufs=1, name=f"oa_{i}")
                pb = ps3p.tile([128, TILE], F32, tag="ob", bufs=1, name=f"ob_{i}")
                # interleave the two PSUM banks so no matmul accumulates
                # into the bank written by the immediately preceding one;
                # pb's weight block is zero-padded 192->256 so it is a
                # full-array matmul too (rows 64:128 of pb are zeros)
                for k in range(3):
                    nc.tensor.matmul(
                        pa[:, :wd],
                        w3_blk(e, k, 0),
                        h2[k][:, :wd],
                        start=(k == 0),
                        stop=(k == 2),
                    )
                    nc.tensor.matmul(
                        pb[:, :wd],
                        w3_blk(e, k, 1),
                        h2[k][:, :wd],
                        start=(k == 0),
                        stop=(k == 2),
                    )
                if chunk_of[i] == i:
                    oseg["o"] = opool.tile(
                        [128, 2, chunk_max], BF16, tag="os", bufs=3, name=f"os_{i}"
                    )
                os = oseg["o"]
                o = s - chunk_start[i]
                lastp = i + 1 == nt
                if lastp:
                    # last tile of the program: pa copies on the (now idle)
                    # scalar engine while the vector engine copies pb, and
                    # the two flush halves ride different queues, to
                    # shorten the copy->flush->drain serial tail
                    nc.scalar.copy(os[:, 0, o : o + wd], pa[:, :wd])
                    nc.vector.tensor_copy(os[0:64, 1, o : o + wd], pb[0:64, :wd])
                else:
                    nc.vector.tensor_copy(os[:, 0, o : o + wd], pa[:, :wd])
                    nc.vector.tensor_copy(os[0:64, 1, o : o + wd], pb[0:64, :wd])
                if i + 1 == nt or chunk_of[i + 1] != chunk_of[i]:
                    lo, hi = chunk_start[i], chunk_end[i]
                    # out flushes ride the lightly-loaded sync queue so the
                    # gpsimd drain chain only carries the early weight DMAs
                    if lastp:
                        nc.sync.dma_start(
                            yp_out[:, 0, lo:hi], os[:, 0, : hi - lo]
                        )
                        nc.scalar.dma_start(
                            yp_out[:, 1, lo:hi], os[:, 1, : hi - lo]
                        )
                    else:
                        nc.sync.dma_start(
                            yp_out[:, :, lo:hi], os[:, :, : hi - lo]
                        )

            def load_w(e):
                nc.gpsimd.dma_start(w_all[:, e], wall_in[e])

            # Startup DMA routing, ordered for the first tile's critical
            # path: expert 0's w1 pieces ride the sync queue right behind
            # the (small) first x piece; the larger w2/w3 pieces go on
            # gpsimd in parallel; the rest of the first x tile rides the
            # scalar queue. Biases are needed only by the first ACTIVATE.
            # The sync queue is the x-tile lifeline: the early x tiles are
            # consumed just-in-time, so nothing else may ride ahead of
            # them (measured: +96KB inserted before xs1 costs ~9us of
            # head stalls). Expert-0 weights go on gpsimd in 4 pieces --
            # more pieces lose more to the ~0.65us SWDGE per-DMA issue
            # overhead at the cold start.
            xs_cur = load_x(0, split=True)
            nc.sync.dma_start(b12_sb[:], b12_in[:])
            xs_next = load_x(1) if nt > 1 else None
            for a, b in [(0, HID), (HID, W2OFF), (W2OFF, W3OFF), (W3OFF, WCOLS)]:
                nc.gpsimd.dma_start(w_all[:, 0, a:b], wall_in[0, :, a:b])
            if E > 1:
                load_w(1)
            # warmup chain (no data deps beyond the memset above); the
            # dummy activation forces the gelu table load early
            wps = ps3p.tile([128, TILE], F32, tag="oa", bufs=1, name="warm_ps")
            nc.scalar.activation(dscr[:], dummy_f[:], AF.Gelu)
            for _ in range(WARMUP_MMS):
                nc.tensor.matmul(
                    wps[:], dummy[:, 0:128], dummy[:, 128:], start=True, stop=True
                )
            h1_cur = l1(0, xs_cur)
            for i in range(nt):
                if i and seq[i][0] != seq[i - 1][0]:
                    nxt = seq[i][0] + 1
                    if nxt < E:
                        load_w(nxt)
                h2 = l2(i, h1_cur)
                if i + 1 < nt:
                    h1_cur = l1(i + 1, xs_next)
                    xs_next = load_x(i + 2) if i + 2 < nt else None
                l3(i, h2)

    if compile:
        nc.compile()
    return nc


def _get_nc(caps):
    key = tuple(caps)
    if key not in _nc_cache:
        _nc_cache[key] = _build(key)
    return _nc_cache[key]


def _route(router_input, router_W, router_b):
    """Replicate reference _gates selection: top-2 by value, 2-way softmax."""
    r = (
        np.asarray(router_input, np.float32)
        .transpose(1, 0, 2, 3)
        .reshape(R_C, NPIX)
    )
    lt = (np.asarray(router_W, np.float32) @ r).T + np.asarray(
        router_b, np.float32
    )[None, :]
    ar = np.arange(NPIX)
    i1 = np.argmax(lt, axis=1)
    l1v = lt[ar, i1]
    ltm = lt.copy()
    ltm[ar, i1] = -np.inf
    i2 = np.argmax(ltm, axis=1)
    l2v = lt[ar, i2]
    e2 = np.exp(l2v - l1v)
    g1 = (1.0 / (1.0 + e2)).astype(np.float32)
    g2 = (e2 / (1.0 + e2)).astype(np.float32)
    return i1, i2, g1, g2


def _plan(i1, i2):
    """Pack (pixel, expert) assignments into per-core per-expert segments.

    Returns caps (per-expert capacity), sl_pix
    [N_CORES, nslot] gather map (pixel index per slot, 0 for padding), and
    M [NPIX, E] with the global flat slot id (core*nslot + slot) of each
    real assignment.
    """
    pe_list, sizes_list = [], []
    caps = []
    for e in range(E):
        pe = np.flatnonzero((i1 == e) | (i2 == e))
        n = len(pe)
        base, r = divmod(n, N_CORES)
        sizes = [base + 1] * r + [base] * (N_CORES - r)
        # max chunk size rounded up to even (fp32r matmul free-dim
        # restriction); floor 256 keeps every tile >=256 wide
        caps.append(max(256, (max(sizes) + 1) & ~1))
        pe_list.append(pe)
        sizes_list.append(sizes)
    nslot = sum(caps)
    offs = np.concatenate([[0], np.cumsum(caps)])[:E]
    sl_pix = np.zeros((N_CORES, nslot), np.int64)
    M = np.zeros((NPIX, E), np.int64)
    for e in range(E):
        pe, sizes = pe_list[e], sizes_list[e]
        start = 0
        for c in range(N_CORES):
            chunk = pe[start : start + sizes[c]]
            start += sizes[c]
            sl_pix[c, offs[e] : offs[e] + len(chunk)] = chunk
            M[chunk, e] = c * nslot + offs[e] + np.arange(len(chunk))
    return caps, sl_pix, M


def kernel(x, router_input, router_W, router_b, W1, b1, W2, b2, W3, b3, **run_kwargs):
    f = np.float32
    i1, i2, g1, g2 = _route(router_input, router_W, router_b)
    caps, sl_pix, M = _plan(i1, i2)
    nc = _get_nc(caps)

    x_flat = np.asarray(x, f).transpose(1, 0, 2, 3).reshape(IN_C, NPIX)
    w1T = np.transpose(np.asarray(W1, f), (0, 2, 1))  # [E, IN_C, HID]
    w1t = np.zeros((E, 128, 2, HID), f)
    w1t[:, :, 0, :] = w1T[:, 0:128, :]
    w1t[:, 0:64, 1, :] = w1T[:, 128:IN_C, :]
    w1t = w1t.astype(ml_dtypes.bfloat16)
    w2t = np.transpose(np.asarray(W2, f), (0, 2, 1))
    w2t = np.ascontiguousarray(
        w2t.reshape(E, 3, 128, HID).transpose(0, 2, 1, 3)
    ).astype(ml_dtypes.bfloat16)
    w3t = np.transpose(np.asarray(W3, f), (0, 2, 1))
    w3t = np.ascontiguousarray(
        w3t.reshape(E, 3, 128, OUT_C).transpose(0, 2, 1, 3)
    )
    w3p = np.zeros((E, 128, 3, 256), np.float32)
    w3p[:, :, :, 0:OUT_C] = w3t
    w3p = w3p.astype(ml_dtypes.bfloat16)
    b1t = np.asarray(b1, f).reshape(E, 3, 128).transpose(2, 0, 1).reshape(128, E * 3)
    b2t = np.asarray(b2, f).reshape(E, 3, 128).transpose(2, 0, 1).reshape(128, E * 3)
    b12t = np.ascontiguousarray(np.stack([b1t, b2t], axis=1))
    wall = np.ascontiguousarray(
        np.concatenate(
            [
                w1t.reshape(E, 128, 2 * HID),
                w2t.reshape(E, 128, 3 * HID),
                w3p.reshape(E, 128, 3 * 256),
            ],
            axis=2,
        )
    )

    nslot = sum(caps)
    in_maps = []
    for c in range(N_CORES):
        xg = x_flat[:, sl_pix[c]]
        xp = np.zeros((128, 2, nslot), ml_dtypes.bfloat16)
        xp[:, 0, :] = xg[0:128]
        xp[0:64, 1, :] = xg[128:IN_C]
        xp[64:128, 1, :] = xg[128:IN_C]
        in_maps.append(
            {
                "xp": xp,
                "wall": wall,
                "b12t": b12t,
            }
        )

    res = run_bass_kernel_spmd(nc, in_maps, list(range(N_CORES)), **run_kwargs)

    # yp[:, 0] = channels 0:128; yp[0:64, 1] = channels 128:192
    yp_all = np.concatenate(
        [res.results[c]["yp"] for c in range(N_CORES)], axis=2
    ).astype(f)
    yp192 = np.concatenate([yp_all[:, 0, :], yp_all[0:64, 1, :]], axis=0)
    ar = np.arange(NPIX)
    j1 = M[ar, i1]
    j2 = M[ar, i2]
    b3f = np.asarray(b3, f)
    out_flat = (
        yp192[:, j1] * g1[None, :]
        + yp192[:, j2] * g2[None, :]
        + b3f[i1].T * g1[None, :]
        + b3f[i2].T * g2[None, :]
    )
    full = np.ascontiguousarray(
        out_flat.reshape(OUT_C, B, H, W).transpose(1, 0, 2, 3).astype(f)
    )
    if run_kwargs:
        kernel.last_results = res
    return full


# revision 70
# speedup vs baseline: 1.0173x; 1.0009x over previous
"""MoE update-MLP Trainium2 kernel (8-core SPMD, sparse top-2 expert compute).

Problem: x (4,192,128,128); a per-pixel router picks top-2 of 8 experts; each
expert is a 3-layer 1x1-conv MLP (192->384 gelu ->384 gelu ->192); output is
the gate-weighted sum over experts.

Strategy: the router is a tiny K=8 linear layer (0.005% of the FLOPs) --
computed on the host, which packs only the top-2 (pixel, expert) assignments
into per-core, per-expert contiguous segments (capacity = largest per-core
chunk, rounded up to even). Each of the 8 cores runs a pure dense GEMM stack
over its ~16.4k assigned pixel-slots (vs 65.5k expert-pixel pairs dense).
The host then applies gates and scatter-adds each pixel's two expert outputs
(plus the gated b3 term) into the full output.

Everything streams in bf16 (x, W1-3, h1, h2, staged outputs; fp32 PSUM
accumulation) -- measured rel err 4.2e-3 vs the 2e-2 budget. Per 512-wide
pixel tile the schedule is 21 uniform full-array 128x128 matmuls (L1: 2
K-chunks x 3 M-chunks with the K=192 contraction zero-padded to 256; L2:
3x3; L3: 3 K-chunks x 2 M-chunks with OUT_C zero-padded 192->256).
Partial-array (tile_position row/col-tiled) variants were measured SLOWER
overall: a tiled LDWEIGHTS cannot preload into the PE background weight
buffer, so every tiled<->full boundary costs ~95-230ns, cancelling the
saved slots; uniform full-array matmuls keep the LDWEIGHTS pipeline
perfect at ~216ns/MM (N=512 streaming rate).

Measured fixed overheads handled explicitly:
- ~6.6us framework preamble; a ~4us chain of dummy matmuls on a memset
  SBUF tile right after it keeps the PE busy through the HAM activity
  window while the first DMAs land, so real matmuls start at the warm
  2.4 GHz clock (cold clock is 1.2 GHz for the first ~3.4us of activity,
  and the first data cannot arrive before ~10us).
- A dummy fp32-input gelu activation pulls the ~2x1.3us ACT_TABLE_LOAD
  off the critical path. Its table transfers ride the scalar HWDGE ring,
  so no startup-critical DMA may use nc.scalar.dma_start.
- Startup DMAs: x tiles + biases on the sync ring, expert-0 weights in
  pieces (first-needed blocks first) + later experts (one DMA each, a
  full segment ahead) on the gpsimd ring.
- Outputs stage into per-flush-chunk SBUF tiles (bufs=3) so tensor_copies
  never wait on an in-flight flush DMA reading the same tile; chunks are
  half-segments, the last expert flushes per tile and orders its narrow
  tile last, and the final tile's two copies/flushes split across the
  scalar+vector engines and scalar+sync rings to shorten the
  copy->flush->drain serial tail.

Software pipeline per tile i: [L2(i) -> gelu] [L1(i+1) -> gelu] [L3(i) ->
copy], with x loads 2 tiles ahead, so ACT latency hides under PE work.
PSUM: 3 (L1) + 3 (L2) + 2 (L3) banks = 8.
"""

import ml_dtypes
import numpy as np

import concourse.bacc as bacc
import concourse.mybir as mybir
import concourse.tile as tile
from concourse.bass_utils import run_bass_kernel_spmd

F32 = mybir.dt.float32
F32R = mybir.dt.float32r
BF16 = mybir.dt.bfloat16
AF = mybir.ActivationFunctionType

N_CORES = 8
B, IN_C, H, W = 4, 192, 128, 128
R_C, E, HID, OUT_C = 8, 8, 384, 192
NPIX = B * H * W
TILE = 512
WARMUP_MMS = 9

_nc_cache: dict = {}


def _tile_seq(caps):
    """[(expert, col_start, width)] covering each expert's capacity segment.

    Tiles are 512 wide except for an odd tail kept >=256 (fp32r matmuls
    below free-dim 256 run at 1/4 rate); a tail under 256 borrows from the
    last full tile. Narrow tiles lead each segment so the very first tile
    of the program needs the least DMA before compute can start -- except
    the last expert, whose narrow tile goes last so the final output flush
    (which the end-of-program drain waits on) is as small as possible.
    """
    seq, off = [], 0
    for e, cap in enumerate(caps):
        k, t = divmod(cap, TILE)
        if t == 0:
            widths = [TILE] * k
        elif t >= 256 or k == 0:
            widths = [max(t, 256)] + [TILE] * k
        else:
            a = (((TILE + t) // 2) + 1) & ~1
            widths = [a, TILE + t - a] + [TILE] * (k - 1)
        if e == E - 1:
            widths = widths[::-1]
        o = 0
        for w in widths:
            seq.append((e, off + o, w))
            o += w
        off += cap
    return seq


def _build(caps, compile: bool = True):
    nslot = sum(caps)
    nc = bacc.Bacc("TRN2", target_bir_lowering=False, debug=False)

    # x and W1 stream in bf16 (the verifier requires both matmul operands
    # to match when either is f32/f32r): halves the largest DMA stream at
    # the same PE rate, with fp32 PSUM accumulation. Only layer 1's inputs
    # are rounded (~0.4%), well inside the 2e-2 budget. Slab 1 of xp/w1t
    # holds channels 128:192 in partitions 0:64 AND replicated in 64:128,
    # so the K=64 remainder matmuls can be row-tiled to either half.
    # w1 | w2 | w3 concatenated, one row per expert; w3's 192 output
    # channels are padded per k-chunk to 2x128 (zero cols) so every matmul
    # is a full-array 128x128 (partial-array LDWEIGHTS cannot preload into
    # the background weight buffer and costs ~95ns per group boundary)
    WCOLS = 2 * HID + 3 * HID + 3 * 256
    W2OFF = 2 * HID
    W3OFF = 2 * HID + 3 * HID
    xp_in = nc.declare_dram_parameter("xp", [128, 2, nslot], BF16, isOutput=False)
    wall_in = nc.declare_dram_parameter("wall", [E, 128, WCOLS], BF16, isOutput=False)
    b12_in = nc.declare_dram_parameter("b12t", [128, 2, E * 3], F32, isOutput=False)
    # output staged and stored as bf16: halves the out DMA stream and the
    # final flush the end-of-program drain waits on; the host upcasts.
    # yp[:, 0] = out channels 0:128; yp[:, 1] rows 0:64 / 64:128 hold the
    # two col-tiled partial sums of channels 128:192 (host adds them).
    yp_out = nc.declare_dram_parameter("yp", [128, 2, nslot], BF16, isOutput=True)

    seq = _tile_seq(caps)
    nt = len(seq)

    with tile.TileContext(nc) as tc:
        with (
            tc.tile_pool(name="wpool", bufs=1) as wpool,
            tc.tile_pool(name="xpool", bufs=4) as xpool,
            tc.tile_pool(name="hpool", bufs=6) as hpool,
            tc.tile_pool(name="psp", bufs=3, space="PSUM") as psp,
        ):
            opool = hpool
            ps1p = ps2p = ps3p = psp
            b12_sb = wpool.tile([128, 2, E * 3], F32)
            b1_sb = b12_sb[:, 0]
            b2_sb = b12_sb[:, 1]
            w_all = wpool.tile([128, E, WCOLS], BF16)

            def w1_blk(e, k, m):  # [128, 128] k-chunk x m-chunk of W1^T
                return w_all[:, e, k * HID + 128 * m : k * HID + 128 * (m + 1)]

            def w2_blk(e, k, m):
                return w_all[
                    :, e, W2OFF + k * HID + 128 * m : W2OFF + k * HID + 128 * (m + 1)
                ]

            def w3_blk(e, k, m):  # m=0: out 0:128; m=1: out 128:192 + 64 zero
                return w_all[
                    :, e, W3OFF + k * 256 + 128 * m : W3OFF + k * 256 + 128 * (m + 1)
                ]

            # PE warmup: a chain of matmuls on a zeroed SBUF tile with no
            # DMA dependency keeps the PE busy right after the framework
            # preamble, so the HAM clock gate is at 8/8 by the time the
            # first data arrives. A dummy activation pulls the ~1.3us gelu
            # ACT_TABLE_LOAD off the critical path too.
            dummy = wpool.tile([128, 128 + TILE], BF16)
            dummy_f = wpool.tile([128, 8], F32)
            dscr = wpool.tile([128, 8], BF16)
            nc.vector.memset(dummy[:], 0.0)
            nc.vector.memset(dummy_f[:], 0.0)

            def load_x(i, split=False):
                _, s, wd = seq[i]
                xs = xpool.tile([128, 2, TILE], BF16, tag="xs", name=f"xs_{i}")
                if split:
                    # chunk 0 lands first so the first three matmuls (which
                    # only need channels 0:128) start as early as possible.
                    # All on the sync queue: the scalar queue is polluted by
                    # the ~1.3us gelu ACT_TABLE_LOAD transfers early on.
                    nc.sync.dma_start(xs[:, 0, :wd], xp_in[:, 0, s : s + wd])
                    nc.sync.dma_start(xs[:, 1, :wd], xp_in[:, 1, s : s + wd])
                else:
                    nc.sync.dma_start(xs[:, :, :wd], xp_in[:, :, s : s + wd])
                return xs

            def l1(i, xs):
                e, _, wd = seq[i]
                h1 = []
                pss = [
                    ps1p.tile([128, TILE], F32, tag="ps1", name=f"ps1_{i}_{m}")
                    for m in range(3)
                ]
                # wavefront order (m0's bank completes at MM 3, not 4, so
                # its gelu can start a slot earlier); K remainder (k=1) is
                # channels 128:192 + 64 zero weight rows -- uniform
                # full-array MMs keep the LDWEIGHTS pipeline perfect. No
                # two consecutive MMs accumulate into the same bank.
                for k, m in [(0, 0), (0, 1), (1, 0), (0, 2), (1, 1), (1, 2)]:
                    nc.tensor.matmul(
                        pss[m][:, :wd],
                        w1_blk(e, k, m),
                        xs[:, k, :wd],
                        start=(k == 0),
                        stop=(k == 1),
                    )
                for m in range(3):
                    hm = hpool.tile([128, TILE], BF16, tag="h1", name=f"h1_{i}_{m}")
                    nc.scalar.activation(
                        hm[:, :wd],
                        pss[m][:, :wd],
                        AF.Gelu,
                        bias=b1_sb[:, 3 * e + m : 3 * e + m + 1],
                    )
                    h1.append(hm)
                return h1

            def l2(i, h1):
                e, _, wd = seq[i]
                pss = [
                    ps2p.tile([128, TILE], F32, tag="ps2", name=f"ps2_{i}_{m}")
                    for m in range(3)
                ]
                # wavefront order: m0's bank finishes its k-accumulation at
                # MM 6 (vs 7 in k-outer order) so its gelu starts earlier;
                # no two consecutive MMs hit the same bank
                for k, m in [
                    (0, 0), (0, 1), (1, 0), (0, 2), (1, 1),
                    (2, 0), (1, 2), (2, 1), (2, 2),
                ]:
                    nc.tensor.matmul(
                        pss[m][:, :wd],
                        w2_blk(e, k, m),
                        h1[k][:, :wd],
                        start=(k == 0),
                        stop=(k == 2),
                    )
                h2 = []
                for m in range(3):
                    hm = hpool.tile([128, TILE], BF16, tag="h2", name=f"h2_{i}_{m}")
                    nc.scalar.activation(
                        hm[:, :wd],
                        pss[m][:, :wd],
                        AF.Gelu,
                        bias=b2_sb[:, 3 * e + m : 3 * e + m + 1],
                    )
                    h2.append(hm)
                return h2

            # flush chunks: tiles are staged into per-chunk SBUF tiles and
            # flushed when the chunk completes, so later tensor_copies never
            # wait on an in-flight flush DMA reading the same tile. Chunks
            # split each expert's segment at the half-capacity tile; the
            # last expert flushes per tile so the end-of-program drain only
            # waits on one small transfer.
            chunk_of, chunk_start, chunk_end = {}, {}, {}
            for e0 in range(E):
                tiles_e = [j for j in range(nt) if seq[j][0] == e0]
                groups = []
                if e0 == E - 1:
                    groups = [[j] for j in tiles_e]
                else:
                    cur = []
                    for j in tiles_e:
                        cur.append(j)
                        _, s0, wd0 = seq[j]
                        off0 = seq[tiles_e[0]][1]
                        if s0 + wd0 - off0 >= caps[e0] // 2 and len(groups) == 0:
                            groups.append(cur)
                            cur = []
                    if cur:
                        groups.append(cur)
                for g in groups:
                    st = seq[g[0]][1]
                    en = seq[g[-1]][1] + seq[g[-1]][2]
                    for j in g:
                        chunk_of[j] = g[0]
                        chunk_start[j] = st
                        chunk_end[j] = en
            chunk_max = max(
                chunk_end[j] - chunk_start[j] for j in range(nt)
            )
            oseg = {"o": None}

            def l3(i, h2):
                e, s, wd = seq[i]
                pa = ps3p.tile([128, TILE], F32, tag="oa", bufs=1, name=f"oa_{i}")
                pb = ps3p.tile([128, TILE], F32, tag="ob", bufs=1, name=f"ob_{i}")
                # interleave the two PSUM banks so no matmul accumulates
                # into the bank written by the immediately preceding one;
                # pb's weight block is zero-padded 192->256 so it is a
                # full-array matmul too (rows 64:128 of pb are zeros)
                for k in range(3):
                    nc.tensor.matmul(
                        pa[:, :wd],
                        w3_blk(e, k, 0),
                        h2[k][:, :wd],
                        start=(k == 0),
                        stop=(k == 2),
                    )
                    nc.tensor.matmul(
                        pb[:, :wd],
                        w3_blk(e, k, 1),
                        h2[k][:, :wd],
                        start=(k == 0),
                        stop=(k == 2),
                    )
                if chunk_of[i] == i:
                    oseg["o"] = opool.tile(
                        [128, 2, chunk_max], BF16, tag="os", bufs=3, name=f"os_{i}"
                    )
                os = oseg["o"]
                o = s - chunk_start[i]
                lastp = i + 1 == nt
                if lastp:
                    # last tile of the program: pa copies on the (now idle)
                    # scalar engine while the vector engine copies pb, and
                    # the two flush halves ride different queues, to
                    # shorten the copy->flush->drain serial tail
                    nc.scalar.copy(os[:, 0, o : o + wd], pa[:, :wd])
                    nc.vector.tensor_copy(os[0:64, 1, o : o + wd], pb[0:64, :wd])
                else:
                    nc.vector.tensor_copy(os[:, 0, o : o + wd], pa[:, :wd])
                    nc.vector.tensor_copy(os[0:64, 1, o : o + wd], pb[0:64, :wd])
                if i + 1 == nt or chunk_of[i + 1] != chunk_of[i]:
                    lo, hi = chunk_start[i], chunk_end[i]
                    # out flushes ride the lightly-loaded sync queue so the
                    # gpsimd drain chain only carries the early weight DMAs
                    if lastp:
                        nc.sync.dma_start(
                            yp_out[:, 0, lo:hi], os[:, 0, : hi - lo]
                        )
                        nc.scalar.dma_start(
                            yp_out[:, 1, lo:hi], os[:, 1, : hi - lo]
                        )
                    else:
                        nc.sync.dma_start(
                            yp_out[:, :, lo:hi], os[:, :, : hi - lo]
                        )

            def load_w(e):
                nc.gpsimd.dma_start(w_all[:, e], wall_in[e])

            # Startup DMA routing, ordered for the first tile's critical
            # path: expert 0's w1 pieces ride the sync queue right behind
            # the (small) first x piece; the larger w2/w3 pieces go on
            # gpsimd in parallel; the rest of the first x tile rides the
            # scalar queue. Biases are needed only by the first ACTIVATE.
            # The sync queue is the x-tile lifeline: the early x tiles are
            # consumed just-in-time, so nothing else may ride ahead of
            # them (measured: +96KB inserted before xs1 costs ~9us of
            # head stalls). Expert-0 weights go on gpsimd in 4 pieces --
            # more pieces lose more to the ~0.65us SWDGE per-DMA issue
            # overhead at the cold start.
            xs_cur = load_x(0, split=True)
            nc.sync.dma_start(b12_sb[:], b12_in[:])
            xs_next = load_x(1) if nt > 1 else None
            for a, b in [(0, HID), (HID, W2OFF), (W2OFF, W3OFF), (W3OFF, WCOLS)]:
                nc.gpsimd.dma_start(w_all[:, 0, a:b], wall_in[0, :, a:b])
            if E > 1:
                load_w(1)
            # warmup chain (no data deps beyond the memset above); the
            # dummy activation forces the gelu table load early
            wps = ps3p.tile([128, TILE], F32, tag="oa", bufs=1, name="warm_ps")
            nc.scalar.activation(dscr[:], dummy_f[:], AF.Gelu)
            for _ in range(WARMUP_MMS):
                nc.tensor.matmul(
                    wps[:], dummy[:, 0:128], dummy[:, 128:], start=True, stop=True
                )
            h1_cur = l1(0, xs_cur)
            for i in range(nt):
                if i and seq[i][0] != seq[i - 1][0]:
                    nxt = seq[i][0] + 1
                    if nxt < E:
                        load_w(nxt)
                h2 = l2(i, h1_cur)
                if i + 1 < nt:
                    h1_cur = l1(i + 1, xs_next)
                    xs_next = load_x(i + 2) if i + 2 < nt else None
                l3(i, h2)

    if compile:
        nc.compile()
    return nc


def _get_nc(caps):
    key = tuple(caps)
    if key not in _nc_cache:
        _nc_cache[key] = _build(key)
    return _nc_cache[key]


def _route(router_input, router_W, router_b):
    """Replicate reference _gates selection: top-2 by value, 2-way softmax."""
    r = (
        np.asarray(router_input, np.float32)
        .transpose(1, 0, 2, 3)
        .reshape(R_C, NPIX)
    )
    lt = (np.asarray(router_W, np.float32) @ r).T + np.asarray(
        router_b, np.float32
    )[None, :]
    ar = np.arange(NPIX)
    i1 = np.argmax(lt, axis=1)
    l1v = lt[ar, i1]
    ltm = lt.copy()
    ltm[ar, i1] = -np.inf
    i2 = np.argmax(ltm, axis=1)
    l2v = lt[ar, i2]
    e2 = np.exp(l2v - l1v)
    g1 = (1.0 / (1.0 + e2)).astype(np.float32)
    g2 = (e2 / (1.0 + e2)).astype(np.float32)
    return i1, i2, g1, g2


def _plan(i1, i2):
    """Pack (pixel, expert) assignments into per-core per-expert segments.

    Returns caps (per-expert capacity), sl_pix
    [N_CORES, nslot] gather map (pixel index per slot, 0 for padding), and
    M [NPIX, E] with the global flat slot id (core*nslot + slot) of each
    real assignment.
    """
    pe_list, sizes_list = [], []
    caps = []
    for e in range(E):
        pe = np.flatnonzero((i1 == e) | (i2 == e))
        n = len(pe)
        base, r = divmod(n, N_CORES)
        sizes = [base + 1] * r + [base] * (N_CORES - r)
        # max chunk size rounded up to even (fp32r matmul free-dim
        # restriction); floor 256 keeps every tile >=256 wide
        caps.append(max(256, (max(sizes) + 1) & ~1))
        pe_list.append(pe)
        sizes_list.append(sizes)
    nslot = sum(caps)
    offs = np.concatenate([[0], np.cumsum(caps)])[:E]
    sl_pix = np.zeros((N_CORES, nslot), np.int64)
    M = np.zeros((NPIX, E), np.int64)
    for e in range(E):
        pe, sizes = pe_list[e], sizes_list[e]
        start = 0
        for c in range(N_CORES):
            chunk = pe[start : start + sizes[c]]
            start += sizes[c]
            sl_pix[c, offs[e] : offs[e] + len(chunk)] = chunk
            M[chunk, e] = c * nslot + offs[e] + np.arange(len(chunk))
    return caps, sl_pix, M


def kernel(x, router_input, router_W, router_b, W1, b1, W2, b2, W3, b3, **run_kwargs):
    f = np.float32
    i1, i2, g1, g2 = _route(router_input, router_W, router_b)
    caps, sl_pix, M = _plan(i1, i2)
    nc = _get_nc(caps)

    x_flat = np.asarray(x, f).transpose(1, 0, 2, 3).reshape(IN_C, NPIX)
    w1T = np.transpose(np.asarray(W1, f), (0, 2, 1))  # [E, IN_C, HID]
    w1t = np.zeros((E, 128, 2, HID), f)
    w1t[:, :, 0, :] = w1T[:, 0:128, :]
    w1t[:, 0:64, 1, :] = w1T[:, 128:IN_C, :]
    w1t = w1t.astype(ml_dtypes.bfloat16)
    w2t = np.transpose(np.asarray(W2, f), (0, 2, 1))
    w2t = np.ascontiguousarray(
        w2t.reshape(E, 3, 128, HID).transpose(0, 2, 1, 3)
    ).astype(ml_dtypes.bfloat16)
    w3t = np.transpose(np.asarray(W3, f), (0, 2, 1))
    w3t = np.ascontiguousarray(
        w3t.reshape(E, 3, 128, OUT_C).transpose(0, 2, 1, 3)
    )
    w3p = np.zeros((E, 128, 3, 256), np.float32)
    w3p[:, :, :, 0:OUT_C] = w3t
    w3p = w3p.astype(ml_dtypes.bfloat16)
    b1t = np.asarray(b1, f).reshape(E, 3, 128).transpose(2, 0, 1).reshape(128, E * 3)
    b2t = np.asarray(b2, f).reshape(E, 3, 128).transpose(2, 0, 1).reshape(128, E * 3)
    b12t = np.ascontiguousarray(np.stack([b1t, b2t], axis=1))
    wall = np.ascontiguousarray(
        np.concatenate(
            [
                w1t.reshape(E, 128, 2 * HID),
                w2t.reshape(E, 128, 3 * HID),
                w3p.reshape(E, 128, 3 * 256),
            ],
            axis=2,
        )
    )

    nslot = sum(caps)
    in_maps = []
    for c in range(N_CORES):
        xg = x_flat[:, sl_pix[c]]
        xp = np.zeros((128, 2, nslot), ml_dtypes.bfloat16)
        xp[:, 0, :] = xg[0:128]
        xp[0:64, 1, :] = xg[128:IN_C]
        xp[64:128, 1, :] = xg[128:IN_C]
        in_maps.append(
            {
                "xp": xp,
                "wall": wall,
                "b12t": b12t,
            }
        )

    res = run_bass_kernel_spmd(nc, in_maps, list(range(N_CORES)), **run_kwargs)

    # yp[:, 0] = channels 0:128; yp[0:64, 1] = channels 128:192
    yp_all = np.concatenate(
        [res.results[c]["yp"] for c in range(N_CORES)], axis=2
    ).astype(f)
    yp192 = np.concatenate([yp_all[:, 0, :], yp_all[0:64, 1, :]], axis=0)
    ar = np.arange(NPIX)
    j1 = M[ar, i1]
    j2 = M[ar, i2]
    b3f = np.asarray(b3, f)
    out_flat = (
        yp192[:, j1] * g1[None, :]
        + yp192[:, j2] * g2[None, :]
        + b3f[i1].T * g1[None, :]
        + b3f[i2].T * g2[None, :]
    )
    full = np.ascontiguousarray(
        out_flat.reshape(OUT_C, B, H, W).transpose(1, 0, 2, 3).astype(f)
    )
    if run_kwargs:
        kernel.last_results = res
    return full


# revision 71
# speedup vs baseline: 1.0195x; 1.0021x over previous
"""MoE update-MLP Trainium2 kernel (8-core SPMD, sparse top-2 expert compute).

Problem: x (4,192,128,128); a per-pixel router picks top-2 of 8 experts; each
expert is a 3-layer 1x1-conv MLP (192->384 gelu ->384 gelu ->192); output is
the gate-weighted sum over experts.

Strategy: the router is a tiny K=8 linear layer (0.005% of the FLOPs) --
computed on the host, which packs only the top-2 (pixel, expert) assignments
into per-core, per-expert contiguous segments (capacity = largest per-core
chunk, rounded up to even). Each of the 8 cores runs a pure dense GEMM stack
over its ~16.4k assigned pixel-slots (vs 65.5k expert-pixel pairs dense).
The host then applies gates and scatter-adds each pixel's two expert outputs
(plus the gated b3 term) into the full output.

Everything streams in bf16 (x, W1-3, h1, h2, staged outputs; fp32 PSUM
accumulation) -- measured rel err 4.2e-3 vs the 2e-2 budget. Per 512-wide
pixel tile the schedule is 21 uniform full-array 128x128 matmuls (L1: 2
K-chunks x 3 M-chunks with the K=192 contraction zero-padded to 256; L2:
3x3; L3: 3 K-chunks x 2 M-chunks with OUT_C zero-padded 192->256).
Partial-array (tile_position row/col-tiled) variants were measured SLOWER
overall: a tiled LDWEIGHTS cannot preload into the PE background weight
buffer, so every tiled<->full boundary costs ~95-230ns, cancelling the
saved slots; uniform full-array matmuls keep the LDWEIGHTS pipeline
perfect at ~216ns/MM (N=512 streaming rate).

Measured fixed overheads handled explicitly:
- ~6.6us framework preamble; a ~4us chain of dummy matmuls on a memset
  SBUF tile right after it keeps the PE busy through the HAM activity
  window while the first DMAs land, so real matmuls start at the warm
  2.4 GHz clock (cold clock is 1.2 GHz for the first ~3.4us of activity,
  and the first data cannot arrive before ~10us).
- A dummy fp32-input gelu activation pulls the ~2x1.3us ACT_TABLE_LOAD
  off the critical path. Its table transfers ride the scalar HWDGE ring,
  so no startup-critical DMA may use nc.scalar.dma_start.
- Startup DMAs: x tiles + biases on the sync ring, expert-0 weights in
  pieces (first-needed blocks first) + later experts (one DMA each, a
  full segment ahead) on the gpsimd ring.
- Outputs stage into per-flush-chunk SBUF tiles (bufs=3) so tensor_copies
  never wait on an in-flight flush DMA reading the same tile; chunks are
  half-segments, the last expert flushes per tile and orders its narrow
  tile last, and the final tile's two copies/flushes split across the
  scalar+vector engines and scalar+sync rings to shorten the
  copy->flush->drain serial tail.

Software pipeline per tile i: [L2(i) -> gelu] [L1(i+1) -> gelu] [L3(i) ->
copy], with x loads 2 tiles ahead, so ACT latency hides under PE work.
PSUM: 3 (L1) + 3 (L2) + 2 (L3) banks = 8.
"""

import ml_dtypes
import numpy as np

import concourse.bacc as bacc
import concourse.mybir as mybir
import concourse.tile as tile
from concourse.bass_utils import run_bass_kernel_spmd

F32 = mybir.dt.float32
F32R = mybir.dt.float32r
BF16 = mybir.dt.bfloat16
AF = mybir.ActivationFunctionType

N_CORES = 8
B, IN_C, H, W = 4, 192, 128, 128
R_C, E, HID, OUT_C = 8, 8, 384, 192
NPIX = B * H * W
TILE = 512
WARMUP_MMS = 9

_nc_cache: dict = {}


def _tile_seq(caps):
    """[(expert, col_start, width)] covering each expert's capacity segment.

    Tiles are 512 wide except for an odd tail kept >=256 (fp32r matmuls
    below free-dim 256 run at 1/4 rate); a tail under 256 borrows from the
    last full tile. Narrow tiles lead each segment so the very first tile
    of the program needs the least DMA before compute can start -- except
    the last expert, whose narrow tile goes last so the final output flush
    (which the end-of-program drain waits on) is as small as possible.
    """
    seq, off = [], 0
    for e, cap in enumerate(caps):
        k, t = divmod(cap, TILE)
        if t == 0:
            widths = [TILE] * k
        elif t >= 256 or k == 0:
            widths = [max(t, 256)] + [TILE] * k
        else:
            a = (((TILE + t) // 2) + 1) & ~1
            widths = [a, TILE + t - a] + [TILE] * (k - 1)
        if e == E - 1:
            widths = widths[::-1]
        o = 0
        for w in widths:
            seq.append((e, off + o, w))
            o += w
        off += cap
    return seq


def _build(caps, compile: bool = True):
    nslot = sum(caps)
    nc = bacc.Bacc("TRN2", target_bir_lowering=False, debug=False)

    # x and W1 stream in bf16 (the verifier requires both matmul operands
    # to match when either is f32/f32r): halves the largest DMA stream at
    # the same PE rate, with fp32 PSUM accumulation. Only layer 1's inputs
    # are rounded (~0.4%), well inside the 2e-2 budget. Slab 1 of xp/w1t
    # holds channels 128:192 in partitions 0:64 AND replicated in 64:128,
    # so the K=64 remainder matmuls can be row-tiled to either half.
    # w1 | w2 | w3 concatenated, one row per expert; w3's 192 output
    # channels are padded per k-chunk to 2x128 (zero cols) so every matmul
    # is a full-array 128x128 (partial-array LDWEIGHTS cannot preload into
    # the background weight buffer and costs ~95ns per group boundary)
    WCOLS = 2 * HID + 3 * HID + 3 * 256
    W2OFF = 2 * HID
    W3OFF = 2 * HID + 3 * HID
    xp_in = nc.declare_dram_parameter("xp", [128, 2, nslot], BF16, isOutput=False)
    wall_in = nc.declare_dram_parameter("wall", [E, 128, WCOLS], BF16, isOutput=False)
    b12_in = nc.declare_dram_parameter("b12t", [128, 2, E * 3], F32, isOutput=False)
    # output staged and stored as bf16: halves the out DMA stream and the
    # final flush the end-of-program drain waits on; the host upcasts.
    # yp[:, 0] = out channels 0:128; yp[:, 1] rows 0:64 / 64:128 hold the
    # two col-tiled partial sums of channels 128:192 (host adds them).
    yp_out = nc.declare_dram_parameter("yp", [128, 2, nslot], BF16, isOutput=True)

    seq = _tile_seq(caps)
    nt = len(seq)

    with tile.TileContext(nc) as tc:
        with (
            tc.tile_pool(name="wpool", bufs=1) as wpool,
            tc.tile_pool(name="xpool", bufs=4) as xpool,
            tc.tile_pool(name="hpool", bufs=6) as hpool,
            tc.tile_pool(name="psp", bufs=3, space="PSUM") as psp,
        ):
            opool = hpool
            ps1p = ps2p = ps3p = psp
            b12_sb = wpool.tile([128, 2, E * 3], F32)
            b1_sb = b12_sb[:, 0]
            b2_sb = b12_sb[:, 1]
            w_all = wpool.tile([128, E, WCOLS], BF16)

            def w1_blk(e, k, m):  # [128, 128] k-chunk x m-chunk of W1^T
                return w_all[:, e, k * HID + 128 * m : k * HID + 128 * (m + 1)]

            def w2_blk(e, k, m):
                return w_all[
                    :, e, W2OFF + k * HID + 128 * m : W2OFF + k * HID + 128 * (m + 1)
                ]

            def w3_blk(e, k, m):  # m=0: out 0:128; m=1: out 128:192 + 64 zero
                return w_all[
                    :, e, W3OFF + k * 256 + 128 * m : W3OFF + k * 256 + 128 * (m + 1)
                ]

            # PE warmup: a chain of matmuls on a zeroed SBUF tile with no
            # DMA dependency keeps the PE busy right after the framework
            # preamble, so the HAM clock gate is at 8/8 by the time the
            # first data arrives. A dummy activation pulls the ~1.3us gelu
            # ACT_TABLE_LOAD off the critical path too.
            dummy = wpool.tile([128, 128 + TILE], BF16)
            dummy_f = wpool.tile([128, 8], F32)
            dscr = wpool.tile([128, 8], BF16)
            nc.vector.memset(dummy[:], 0.0)
            nc.vector.memset(dummy_f[:], 0.0)

            def load_x(i, split=False):
                _, s, wd = seq[i]
                xs = xpool.tile([128, 2, TILE], BF16, tag="xs", name=f"xs_{i}")
                if split:
                    # chunk 0 lands first so the first three matmuls (which
                    # only need channels 0:128) start as early as possible.
                    # All on the sync queue: the scalar queue is polluted by
                    # the ~1.3us gelu ACT_TABLE_LOAD transfers early on.
                    nc.sync.dma_start(xs[:, 0, :wd], xp_in[:, 0, s : s + wd])
                    nc.sync.dma_start(xs[:, 1, :wd], xp_in[:, 1, s : s + wd])
                else:
                    nc.sync.dma_start(xs[:, :, :wd], xp_in[:, :, s : s + wd])
                return xs

            def l1(i, xs):
                e, _, wd = seq[i]
                h1 = []
                pss = [
                    ps1p.tile([128, TILE], F32, tag="ps1", name=f"ps1_{i}_{m}")
                    for m in range(3)
                ]
                # wavefront order (m0's bank completes at MM 3, not 4, so
                # its gelu can start a slot earlier); K remainder (k=1) is
                # channels 128:192 + 64 zero weight rows -- uniform
                # full-array MMs keep the LDWEIGHTS pipeline perfect. No
                # two consecutive MMs accumulate into the same bank.
                for k, m in [(0, 0), (0, 1), (1, 0), (0, 2), (1, 1), (1, 2)]:
                    nc.tensor.matmul(
                        pss[m][:, :wd],
                        w1_blk(e, k, m),
                        xs[:, k, :wd],
                        start=(k == 0),
                        stop=(k == 1),
                    )
                for m in range(3):
                    hm = hpool.tile([128, TILE], BF16, tag="h1", name=f"h1_{i}_{m}")
                    nc.scalar.activation(
                        hm[:, :wd],
                        pss[m][:, :wd],
                        AF.Gelu,
                        bias=b1_sb[:, 3 * e + m : 3 * e + m + 1],
                    )
                    h1.append(hm)
                return h1

            def l2(i, h1):
                e, _, wd = seq[i]
                pss = [
                    ps2p.tile([128, TILE], F32, tag="ps2", name=f"ps2_{i}_{m}")
                    for m in range(3)
                ]
                # wavefront order: m0's bank finishes its k-accumulation at
                # MM 5 (vs 7 in k-outer order) so its gelu starts earlier;
                # no two consecutive MMs hit the same bank
                for k, m in [
                    (0, 0), (0, 1), (1, 0), (0, 2), (2, 0),
                    (1, 1), (2, 1), (1, 2), (2, 2),
                ]:
                    nc.tensor.matmul(
                        pss[m][:, :wd],
                        w2_blk(e, k, m),
                        h1[k][:, :wd],
                        start=(k == 0),
                        stop=(k == 2),
                    )
                h2 = []
                for m in range(3):
                    hm = hpool.tile([128, TILE], BF16, tag="h2", name=f"h2_{i}_{m}")
                    nc.scalar.activation(
                        hm[:, :wd],
                        pss[m][:, :wd],
                        AF.Gelu,
                        bias=b2_sb[:, 3 * e + m : 3 * e + m + 1],
                    )
                    h2.append(hm)
                return h2

            # flush chunks: tiles are staged into per-chunk SBUF tiles and
            # flushed when the chunk completes, so later tensor_copies never
            # wait on an in-flight flush DMA reading the same tile. Chunks
            # split each expert's segment at the half-capacity tile; the
            # last expert flushes per tile so the end-of-program drain only
            # waits on one small transfer.
            chunk_of, chunk_start, chunk_end = {}, {}, {}
            for e0 in range(E):
                tiles_e = [j for j in range(nt) if seq[j][0] == e0]
                groups = []
                if e0 == E - 1:
                    groups = [[j] for j in tiles_e]
                else:
                    cur = []
                    for j in tiles_e:
                        cur.append(j)
                        _, s0, wd0 = seq[j]
                        off0 = seq[tiles_e[0]][1]
                        if s0 + wd0 - off0 >= caps[e0] // 2 and len(groups) == 0:
                            groups.append(cur)
                            cur = []
                    if cur:
                        groups.append(cur)
                for g in groups:
                    st = seq[g[0]][1]
                    en = seq[g[-1]][1] + seq[g[-1]][2]
                    for j in g:
                        chunk_of[j] = g[0]
                        chunk_start[j] = st
                        chunk_end[j] = en
            chunk_max = max(
                chunk_end[j] - chunk_start[j] for j in range(nt)
            )
            oseg = {"o": None}

            def l3(i, h2):
                e, s, wd = seq[i]
                pa = ps3p.tile([128, TILE], F32, tag="oa", bufs=1, name=f"oa_{i}")
                pb = ps3p.tile([128, TILE], F32, tag="ob", bufs=1, name=f"ob_{i}")
                # interleave the two PSUM banks so no matmul accumulates
                # into the bank written by the immediately preceding one;
                # pb's weight block is zero-padded 192->256 so it is a
                # full-array matmul too (rows 64:128 of pb are zeros)
                for k in range(3):
                    nc.tensor.matmul(
                        pa[:, :wd],
                        w3_blk(e, k, 0),
                        h2[k][:, :wd],
                        start=(k == 0),
                        stop=(k == 2),
                    )
                    nc.tensor.matmul(
                        pb[:, :wd],
                        w3_blk(e, k, 1),
                        h2[k][:, :wd],
                        start=(k == 0),
                        stop=(k == 2),
                    )
                if chunk_of[i] == i:
                    oseg["o"] = opool.tile(
                        [128, 2, chunk_max], BF16, tag="os", bufs=3, name=f"os_{i}"
                    )
                os = oseg["o"]
                o = s - chunk_start[i]
                lastp = i + 1 == nt
                if lastp:
                    # last tile of the program: pa copies on the (now idle)
                    # scalar engine while the vector engine copies pb, and
                    # the two flush halves ride different queues, to
                    # shorten the copy->flush->drain serial tail
                    nc.scalar.copy(os[:, 0, o : o + wd], pa[:, :wd])
                    nc.vector.tensor_copy(os[0:64, 1, o : o + wd], pb[0:64, :wd])
                else:
                    nc.vector.tensor_copy(os[:, 0, o : o + wd], pa[:, :wd])
                    nc.vector.tensor_copy(os[0:64, 1, o : o + wd], pb[0:64, :wd])
                if i + 1 == nt or chunk_of[i + 1] != chunk_of[i]:
                    lo, hi = chunk_start[i], chunk_end[i]
                    # out flushes ride the lightly-loaded sync queue so the
                    # gpsimd drain chain only carries the early weight DMAs
                    if lastp:
                        nc.sync.dma_start(
                            yp_out[:, 0, lo:hi], os[:, 0, : hi - lo]
                        )
                        nc.scalar.dma_start(
                            yp_out[:, 1, lo:hi], os[:, 1, : hi - lo]
                        )
                    else:
                        nc.sync.dma_start(
                            yp_out[:, :, lo:hi], os[:, :, : hi - lo]
                        )

            def load_w(e):
                nc.gpsimd.dma_start(w_all[:, e], wall_in[e])

            # Startup DMA routing, ordered for the first tile's critical
            # path: expert 0's w1 pieces ride the sync queue right behind
            # the (small) first x piece; the larger w2/w3 pieces go on
            # gpsimd in parallel; the rest of the first x tile rides the
            # scalar queue. Biases are needed only by the first ACTIVATE.
            # The sync queue is the x-tile lifeline: the early x tiles are
            # consumed just-in-time, so nothing else may ride ahead of
            # them (measured: +96KB inserted before xs1 costs ~9us of
            # head stalls). Expert-0 weights go on gpsimd in 4 pieces --
            # more pieces lose more to the ~0.65us SWDGE per-DMA issue
            # overhead at the cold start.
            xs_cur = load_x(0, split=True)
            nc.sync.dma_start(b12_sb[:], b12_in[:])
            xs_next = load_x(1) if nt > 1 else None
            for a, b in [(0, HID), (HID, W2OFF), (W2OFF, W3OFF), (W3OFF, WCOLS)]:
                nc.gpsimd.dma_start(w_all[:, 0, a:b], wall_in[0, :, a:b])
            if E > 1:
                load_w(1)
            # warmup chain (no data deps beyond the memset above); the
            # dummy activation forces the gelu table load early
            wps = ps3p.tile([128, TILE], F32, tag="oa", bufs=1, name="warm_ps")
            nc.scalar.activation(dscr[:], dummy_f[:], AF.Gelu)
            for _ in range(WARMUP_MMS):
                nc.tensor.matmul(
                    wps[:], dummy[:, 0:128], dummy[:, 128:], start=True, stop=True
                )
            h1_cur = l1(0, xs_cur)
            for i in range(nt):
                if i and seq[i][0] != seq[i - 1][0]:
                    nxt = seq[i][0] + 1
                    if nxt < E:
                        load_w(nxt)
                h2 = l2(i, h1_cur)
                if i + 1 < nt:
                    h1_cur = l1(i + 1, xs_next)
                    xs_next = load_x(i + 2) if i + 2 < nt else None
                l3(i, h2)

    if compile:
        nc.compile()
    return nc


def _get_nc(caps):
    key = tuple(caps)
    if key not in _nc_cache:
        _nc_cache[key] = _build(key)
    return _nc_cache[key]


def _route(router_input, router_W, router_b):
    """Replicate reference _gates selection: top-2 by value, 2-way softmax."""
    r = (
        np.asarray(router_input, np.float32)
        .transpose(1, 0, 2, 3)
        .reshape(R_C, NPIX)
    )
    lt = (np.asarray(router_W, np.float32) @ r).T + np.asarray(
        router_b, np.float32
    )[None, :]
    ar = np.arange(NPIX)
    i1 = np.argmax(lt, axis=1)
    l1v = lt[ar, i1]
    ltm = lt.copy()
    ltm[ar, i1] = -np.inf
    i2 = np.argmax(ltm, axis=1)
    l2v = lt[ar, i2]
    e2 = np.exp(l2v - l1v)
    g1 = (1.0 / (1.0 + e2)).astype(np.float32)
    g2 = (e2 / (1.0 + e2)).astype(np.float32)
    return i1, i2, g1, g2


def _plan(i1, i2):
    """Pack (pixel, expert) assignments into per-core per-expert segments.

    Returns caps (per-expert capacity), sl_pix
    [N_CORES, nslot] gather map (pixel index per slot, 0 for padding), and
    M [NPIX, E] with the global flat slot id (core*nslot + slot) of each
    real assignment.
    """
    pe_list, sizes_list = [], []
    caps = []
    for e in range(E):
        pe = np.flatnonzero((i1 == e) | (i2 == e))
        n = len(pe)
        base, r = divmod(n, N_CORES)
        sizes = [base + 1] * r + [base] * (N_CORES - r)
        # max chunk size rounded up to even (fp32r matmul free-dim
        # restriction); floor 256 keeps every tile >=256 wide
        caps.append(max(256, (max(sizes) + 1) & ~1))
        pe_list.append(pe)
        sizes_list.append(sizes)
    nslot = sum(caps)
    offs = np.concatenate([[0], np.cumsum(caps)])[:E]
    sl_pix = np.zeros((N_CORES, nslot), np.int64)
    M = np.zeros((NPIX, E), np.int64)
    for e in range(E):
        pe, sizes = pe_list[e], sizes_list[e]
        start = 0
        for c in range(N_CORES):
            chunk = pe[start : start + sizes[c]]
            start += sizes[c]
            sl_pix[c, offs[e] : offs[e] + len(chunk)] = chunk
            M[chunk, e] = c * nslot + offs[e] + np.arange(len(chunk))
    return caps, sl_pix, M


def kernel(x, router_input, router_W, router_b, W1, b1, W2, b2, W3, b3, **run_kwargs):
    f = np.float32
    i1, i2, g1, g2 = _route(router_input, router_W, router_b)
    caps, sl_pix, M = _plan(i1, i2)
    nc = _get_nc(caps)

    x_flat = np.asarray(x, f).transpose(1, 0, 2, 3).reshape(IN_C, NPIX)
    w1T = np.transpose(np.asarray(W1, f), (0, 2, 1))  # [E, IN_C, HID]
    w1t = np.zeros((E, 128, 2, HID), f)
    w1t[:, :, 0, :] = w1T[:, 0:128, :]
    w1t[:, 0:64, 1, :] = w1T[:, 128:IN_C, :]
    w1t = w1t.astype(ml_dtypes.bfloat16)
    w2t = np.transpose(np.asarray(W2, f), (0, 2, 1))
    w2t = np.ascontiguousarray(
        w2t.reshape(E, 3, 128, HID).transpose(0, 2, 1, 3)
    ).astype(ml_dtypes.bfloat16)
    w3t = np.transpose(np.asarray(W3, f), (0, 2, 1))
    w3t = np.ascontiguousarray(
        w3t.reshape(E, 3, 128, OUT_C).transpose(0, 2, 1, 3)
    )
    w3p = np.zeros((E, 128, 3, 256), np.float32)
    w3p[:, :, :, 0:OUT_C] = w3t
    w3p = w3p.astype(ml_dtypes.bfloat16)
    b1t = np.asarray(b1, f).reshape(E, 3, 128).transpose(2, 0, 1).reshape(128, E * 3)
    b2t = np.asarray(b2, f).reshape(E, 3, 128).transpose(2, 0, 1).reshape(128, E * 3)
    b12t = np.ascontiguousarray(np.stack([b1t, b2t], axis=1))
    wall = np.ascontiguousarray(
        np.concatenate(
            [
                w1t.reshape(E, 128, 2 * HID),
                w2t.reshape(E, 128, 3 * HID),
                w3p.reshape(E, 128, 3 * 256),
            ],
            axis=2,
        )
    )

    nslot = sum(caps)
    in_maps = []
    for c in range(N_CORES):
        xg = x_flat[:, sl_pix[c]]
        xp = np.zeros((128, 2, nslot), ml_dtypes.bfloat16)
        xp[:, 0, :] = xg[0:128]
        xp[0:64, 1, :] = xg[128:IN_C]
        xp[64:128, 1, :] = xg[128:IN_C]
        in_maps.append(
            {
                "xp": xp,
                "wall": wall,
                "b12t": b12t,
            }
        )

    res = run_bass_kernel_spmd(nc, in_maps, list(range(N_CORES)), **run_kwargs)

    # yp[:, 0] = channels 0:128; yp[0:64, 1] = channels 128:192
    yp_all = np.concatenate(
        [res.results[c]["yp"] for c in range(N_CORES)], axis=2
    ).astype(f)
    yp192 = np.concatenate([yp_all[:, 0, :], yp_all[0:64, 1, :]], axis=0)
    ar = np.arange(NPIX)
    j1 = M[ar, i1]
    j2 = M[ar, i2]
    b3f = np.asarray(b3, f)
    out_flat = (
        yp192[:, j1] * g1[None, :]
        + yp192[:, j2] * g2[None, :]
        + b3f[i1].T * g1[None, :]
        + b3f[i2].T * g2[None, :]
    )
    full = np.ascontiguousarray(
        out_flat.reshape(OUT_C, B, H, W).transpose(1, 0, 2, 3).astype(f)
    )
    if run_kwargs:
        kernel.last_results = res
    return full
